# revision 6
# baseline (speedup 1.0000x reference)
"""ChirpletKANLinear forward on 8 Trainium2 NeuronCores.

Math (per reference):
    base_out[b,o]  = sum_i silu(x[b,i]) * BW[o,i]
    xs             = (x[b,i] - T[o,i]) / S[o,i]
    out[b,o]       = base_out + sum_i cos(2pi*F*xs)*exp(-0.5*xs^2)*CW[o,i]
                     + bias[o]

Algorithm: each chirplet atom h_oi(x) = CW*cos(2pi*F*(x-T)/S)*
exp(-0.5*((x-T)/S)^2) is a fixed smooth function of the scalar x on
x in [-a, a].  Expand it in a shared Fourier-cosine basis
    h_oi(x) ~= sum_{k<K} c[k,o,i] * cos(k * pi * (x+a) / (2a)),
with coefficients c (DCT-II of h_oi, host-precomputed from the weight
tensors only).  The envelope exp(-0.5 xs^2) vanishes at the domain ends,
so the even periodic extension is smooth and K ~= 2a*max(2pi*F/S)/pi + 12
features suffice for ~1e-3 accuracy.  The whole layer then becomes
    out[b,o] = sum_i sum_k c[k,o,i] * cos(k*theta(x[b,i]))     (+ base path
               with feature silu(x) and coefficients BW, + bias)
i.e. ONE dense matmul with contraction (i,k), plus K cheap cosine feature
tiles that depend on x alone - no per-(o,i) transcendentals.

On-device per k (proven int-phase-wrap pattern):
    DVE: mf = int32(round(2^18 * (k*(xc+a)/(4a) + (k+1)/4)))   (fp32 affine)
    DVE: w  = mf << 14                  (wraps phase mod 2^32 = mod 2pi)
    ACT: f_k = Sin(w * 2pi/2^32) -> bf16  = cos(k*theta) (+pi/2 offset in
         the (k+1)/4 term turns Sin into Cos)
    PE : psum[oc] += lhsT(c[k, oc-chunk])^T @ f_k   (4 chunks of 128 outs)

Sharding: 8 cores = 4 in_feature groups (128 each) x 2 batch halves
(512 each).  Every core computes a partial (512 out, 512 batch) fp32
product over its 128 in-features; the host sums the 4 partials per batch
half (the unshard step) - bias+DC folded into the g==0 cores on device.
"""

import math

import numpy as np
import ml_dtypes

import concourse.bass as bass
import concourse.bacc as bacc
import concourse.tile as tile
import concourse.mybir as mybir
from concourse.bass_utils import run_bass_kernel_spmd

B, IN, OUT = 1024, 512, 512
NCORES = 8
NG = 4                      # in-feature groups
NH = 2                      # batch halves
IG = IN // NG               # 128 in-features per group
BH = B // NH                # 512 batch per half
NOC = OUT // 128            # 4 output chunks of 128

A = 5.2                     # cosine-series half-range
K = 48                      # features: k=0 -> silu (base path), 1..K-1 cos
NQ = 512                    # DCT quadrature points (host)

F32 = mybir.dt.float32
I32 = mybir.dt.int32
BF16 = mybir.dt.bfloat16
AF = mybir.ActivationFunctionType
ALU = mybir.AluOpType
TWO_PI = 2.0 * math.pi

TRACE = False
LAST_RESULT = None

_nc_cache = None


def _build_nc(loop_r=None):
    nc = bacc.Bacc("TRN2", target_bir_lowering=False, debug=False,
                   num_devices=NCORES)

    xT_d = nc.dram_tensor("xT", [IG, BH], F32, kind="ExternalInput")
    cw_d = nc.dram_tensor("cw", [IG, K, NOC, 128], BF16,
                          kind="ExternalInput")
    bias_d = nc.dram_tensor("biasv", [128, NOC], F32, kind="ExternalInput")
    out_d = nc.dram_tensor("out", [NOC, 128, BH], F32, kind="ExternalOutput")

    with tile.TileContext(nc) as tc:
        with (
            tc.tile_pool(name="singles", bufs=1) as singles,
            tc.tile_pool(name="mfpool", bufs=2) as mfpool,
            tc.tile_pool(name="wpool", bufs=2) as wpool,
            tc.tile_pool(name="fpool", bufs=4) as fpool,
            tc.tile_pool(name="psum", bufs=1,
                         space=bass.MemorySpace.PSUM) as psump,
        ):
            xT_sb = singles.tile([IG, BH], F32)
            nc.sync.dma_start(xT_sb[:], xT_d[:])
            cw_sb = singles.tile([IG, K, NOC, 128], BF16)
            nc.sync.dma_start(cw_sb[:], cw_d[:])
            bias_sb = singles.tile([128, NOC], F32)
            nc.sync.dma_start(bias_sb[:], bias_d[:])
            # clamp to [-A, A] so the periodic basis never sees out-of-range x
            xc_sb = singles.tile([IG, BH], F32)
            nc.vector.tensor_scalar(xc_sb, xT_sb, -A, A, ALU.max, ALU.min)

            psum_acc = psump.tile([128, NOC, BH], F32)

            def compute_body():
                # k = 0: silu(x) feature against base_weight columns
                f0 = fpool.tile([IG, BH], BF16, tag="f")
                nc.scalar.activation(f0, xT_sb, AF.Silu)
                for oc in range(NOC):
                    nc.tensor.matmul(
                        psum_acc[:, oc, :], cw_sb[:, 0, oc, :], f0,
                        start=True, stop=False, skip_group_check=True)

                for k in range(1, K):
                    mf = mfpool.tile([IG, BH], I32, tag="mf")
                    nc.vector.tensor_scalar(
                        mf, xc_sb,
                        float(2 ** 18) * k / (4 * A),
                        float(2 ** 16) * (k + 1),
                        ALU.mult, ALU.add)
                    w = wpool.tile([IG, BH], I32, tag="w")
                    nc.vector.tensor_scalar(
                        w, mf, 14, 0,
                        ALU.arith_shift_left, ALU.arith_shift_right)
                    f = fpool.tile([IG, BH], BF16, tag="f")
                    nc.scalar.activation(f, w, AF.Sin, bias=0.0,
                                         scale=TWO_PI / 2 ** 32)
                    last = k == K - 1
                    for oc in range(NOC):
                        nc.tensor.matmul(
                            psum_acc[:, oc, :], cw_sb[:, k, oc, :], f,
                            start=False, stop=last, skip_group_check=True)

            if loop_r:
                with tc.For_i(0, loop_r, 1,
                              hint_engines=(mybir.EngineType.Activation,
                                            mybir.EngineType.DVE,
                                            mybir.EngineType.PE)):
                    compute_body()
            else:
                compute_body()

            out_sb = singles.tile([128, NOC, BH], F32)
            for oc in range(NOC):
                nc.scalar.activation(out_sb[:, oc, :], psum_acc[:, oc, :],
                                     AF.Identity,
                                     bias=bias_sb[:, oc:oc + 1], scale=1.0)
                nc.sync.dma_start(out_d[oc], out_sb[:, oc, :])

    nc.compile()
    return nc


def _coeffs(inp):
    """DCT-II cosine coefficients c[k, o, i] of the chirplet atoms."""
    f = np.float32(inp["frequency"])
    s = np.float32(inp["scale"])
    t = np.float32(inp["translation"])
    cwt = np.float32(inp["chirplet_weights"])
    thq = ((np.arange(NQ) + 0.5) * (math.pi / NQ)).astype(np.float32)
    xq = (2 * A / math.pi) * thq - A                       # (NQ,)
    basis = np.cos(np.outer(thq, np.arange(K))).astype(np.float32)  # (NQ, K)
    basis *= 2.0 / NQ
    basis[:, 0] *= 0.5
    c = np.empty((K, OUT, IN), np.float32)
    for o0 in range(0, OUT, 64):
        o1 = o0 + 64
        u = (xq[None, None, :] - t[o0:o1, :, None]) / s[o0:o1, :, None]
        h = (np.cos(np.float32(TWO_PI) * f[o0:o1, :, None] * u)
             * np.exp(np.float32(-0.5) * u * u) * cwt[o0:o1, :, None])
        c[:, o0:o1, :] = np.einsum("oiq,qk->koi", h, basis, optimize=True)
    return c


def _host_prep(inp):
    x = np.float32(inp["x"])
    c = _coeffs(inp)                                     # (K, OUT, IN)
    c0sum_v = c[0].sum(axis=1) + np.float32(inp["bias"])  # (OUT,) DC + bias
    c[0] = np.float32(inp["base_weight"])                # k=0 slot: base path
    maps = []
    for g in range(NG):
        isl = slice(g * IG, (g + 1) * IG)
        # cw[p, k, oc, m] = c[k, oc*128+m, g*128+p]
        cw = np.ascontiguousarray(
            c[:, :, isl].transpose(2, 0, 1).reshape(IG, K, NOC, 128)
        ).astype(ml_dtypes.bfloat16)
        for h in range(NH):
            bsl = slice(h * BH, (h + 1) * BH)
            xT = np.ascontiguousarray(x[bsl, isl].T)     # (IG, BH)
            maps.append({"xT": xT, "cw": cw, "biasv": None, "g": g})
    # The cos-series k=0 (DC) slot was replaced by the base path, so its
    # contribution sum_i c0[o,i] plus the bias is added via the biasv
    # vector - on the g==0 cores only (zeros elsewhere to avoid double add).
    for m in maps:
        if m["g"] == 0:
            m["biasv"] = np.ascontiguousarray(
                c0sum_v.reshape(NOC, 128).T.astype(np.float32))  # (128, NOC)
        else:
            m["biasv"] = np.zeros((128, NOC), np.float32)
        del m["g"]
    return maps


def kernel(**inputs):
    global _nc_cache, LAST_RESULT
    np_in = {k: np.asarray(v, dtype=np.float32) for k, v in inputs.items()}
    if _nc_cache is None:
        _nc_cache = _build_nc()
    in_maps = _host_prep(np_in)
    res = run_bass_kernel_spmd(
        _nc_cache, in_maps, core_ids=list(range(NCORES)), trace=TRACE)
    LAST_RESULT = res
    # results[c]: partial (NOC, 128, BH) for core c = (g, h)
    full = np.zeros((B, OUT), np.float32)
    for ci, r in enumerate(res.results):
        g, h = divmod(ci, NH)
        part = np.asarray(r["out"], np.float32).reshape(OUT, BH)
        full[h * BH:(h + 1) * BH, :] += part.T
    return full
